# revision 13
# baseline (speedup 1.0000x reference)
"""CosineDistanceAttention Trainium2 kernel.

Strategy (see spec: data-parallel over B across 8 cores, 2 batches/core):
  sim[l,n] = (x_l . p_n) / (|x_l| |p_n|), masked softmax over n, then
  emb = W @ profile.  Everything on-device is computed in a transposed
  [n, l] / [d, l] layout so that
    - the softmax mask is a per-partition bias fused into the Exp op,
    - the softmax denominator is a PE ones-column reduction,
    - the embedding matmul consumes `profile` in its natural layout,
  and no transpose of the big [L, N] weights tensor is ever needed.
  Outputs leave the device as w_t (B, N, L) and emb_t (B, D, L); the host
  gather step transposes them back to (B, L, N) / (B, L, D).
"""

import sys

if "/opt/trn_rl_repo" not in sys.path:
    sys.path.insert(0, "/opt/trn_rl_repo")

import numpy as np

B, L, N, D = 16, 4096, 512, 256
NCORES = 8
BS = B // NCORES          # batches per core
P128 = 128
NCH = N // P128           # 4 profile chunks
DH = D // P128            # 2 contraction chunks
LSUP = 512                # L super-tile
LCH = LSUP // P128        # 4 l chunks per super-tile
NST = L // LSUP           # 8 super-tiles per batch
NEG = -1.0e30             # additive mask; exp(x + NEG) == 0 in fp32

_compiled = None


def _build():
    import concourse.bacc as bacc
    import concourse.tile as tile
    import concourse.mybir as mybir
    from concourse.masks import make_identity

    f32 = mybir.dt.float32
    Alu = mybir.AluOpType
    Act = mybir.ActivationFunctionType

    nc = bacc.Bacc("TRN2", target_bir_lowering=False, debug=False)

    X = nc.dram_tensor("x", (BS, L, D), f32, kind="ExternalInput").ap()
    PR = nc.dram_tensor("profile", (BS, N, D), f32, kind="ExternalInput").ap()
    MB = nc.dram_tensor("maskb", (BS, P128, NCH), f32, kind="ExternalInput").ap()
    WT = nc.dram_tensor("w_t", (BS, N, L), f32, kind="ExternalOutput").ap()
    ET = nc.dram_tensor("emb_t", (BS, D, L), f32, kind="ExternalOutput").ap()

    with tile.TileContext(nc) as tc:
        with (
            tc.tile_pool(name="singles", bufs=1) as singles,
            tc.tile_pool(name="batch", bufs=2) as batch,
            tc.tile_pool(name="xio", bufs=3) as xio,
            tc.tile_pool(name="wio", bufs=2) as wio,
            tc.tile_pool(name="small", bufs=3) as small,
            tc.tile_pool(name="scratch", bufs=2) as scratch,
            tc.tile_pool(name="ptr", bufs=2, space="PSUM") as ppool_tr,
            tc.tile_pool(name="pdt", bufs=3, space="PSUM") as ppool_dt,
            tc.tile_pool(name="ps", bufs=1, space="PSUM") as ppool_s,
            tc.tile_pool(name="pe", bufs=2, space="PSUM") as ppool_e,
        ):
            i32 = mybir.dt.int32
            ident = singles.tile([P128, P128], f32)
            make_identity(nc, ident)
            ones_col = singles.tile([P128, 1], f32)
            nc.vector.memset(ones_col, 1.0)
            magic = singles.tile([P128, NCH], i32)
            nc.vector.memset(magic, 0x5F3759DF)

            def rnorm(t, k, tagpfx):
                # t: [128, k, D] rows; returns [128, k] of 1/||row||
                n2 = small.tile([P128, k], f32, tag=tagpfx + "n2")
                for i in range(k):
                    sq = scratch.tile([P128, D], f32, tag="sq")
                    # fused square + row-sum on DVE (keeps ACT exp-only)
                    nc.vector.scalar_tensor_tensor(
                        out=sq, in0=t[:, i, :], scalar=1.0, in1=t[:, i, :],
                        op0=Alu.mult, op1=Alu.mult,
                        accum_out=n2[:, i:i + 1],
                    )
                # rsqrt fully on DVE (magic-constant seed + 3 Newton steps)
                # so the ACT table set stays exp-only (table swaps are 1.3us).
                y = small.tile([P128, k], f32, tag=tagpfx + "y")
                nc.vector.tensor_scalar(
                    out=y.bitcast(i32), in0=n2.bitcast(i32),
                    scalar1=1, scalar2=None, op0=Alu.logical_shift_right,
                )
                nc.vector.tensor_sub(
                    out=y.bitcast(i32), in0=magic[:, :k], in1=y.bitcast(i32)
                )
                t1 = small.tile([P128, k], f32, tag=tagpfx + "t1")
                for _ in range(3):
                    # y <- y * (1.5 - 0.5 * n2 * y^2)
                    nc.vector.tensor_mul(out=t1, in0=y, in1=y)
                    nc.vector.scalar_tensor_tensor(
                        out=t1, in0=t1, scalar=-0.5, in1=n2,
                        op0=Alu.mult, op1=Alu.mult,
                    )
                    nc.vector.scalar_tensor_tensor(
                        out=y, in0=t1, scalar=1.5, in1=y,
                        op0=Alu.add, op1=Alu.mult,
                    )
                return y

            def batch_prep(b):
                prof = batch.tile([P128, NCH, D], f32, tag="prof")
                nc.sync.dma_start(
                    out=prof, in_=PR[b].rearrange("(c p) d -> p c d", p=P128)
                )
                mb_sb = batch.tile([P128, NCH], f32, tag="mb")
                nc.sync.dma_start(out=mb_sb, in_=MB[b])
                inv_pn = rnorm(prof, NCH, "p")
                ph = batch.tile([P128, NCH, D], f32, tag="ph")
                for c in range(NCH):
                    nc.vector.tensor_scalar_mul(
                        out=ph[:, c, :], in0=prof[:, c, :],
                        scalar1=inv_pn[:, c:c + 1],
                    )
                phT = batch.tile([P128, DH, N], f32, tag="phT")
                for dh in range(DH):
                    ptr = ppool_tr.tile([P128, NCH, P128], f32, tag="tr")
                    for c in range(NCH):
                        nc.tensor.transpose(
                            ptr[:, c, :],
                            ph[:, c, dh * P128:(dh + 1) * P128], ident,
                        )
                    nc.scalar.copy(
                        phT[:, dh, :], ptr.rearrange("p a b -> p (a b)")
                    )
                return prof, mb_sb, phT

            nextb = batch_prep(0)
            for b in range(BS):
                prof, mb_sb, phT = nextb

                # ---- L super-tiles (software-pipelined: tile n+1's x-prep
                # is issued before tile n's softmax/embed tail so the PE
                # never waits on the DVE prep chain) ----
                def prep(st):
                    L0 = st * LSUP
                    xs = xio.tile([P128, LCH, D], f32, tag="xs")
                    nc.sync.dma_start(
                        out=xs,
                        in_=X[b, L0:L0 + LSUP, :].rearrange(
                            "(lc p) d -> p lc d", p=P128
                        ),
                    )
                    inv_xn = rnorm(xs, LCH, "x")
                    xh = xio.tile([P128, LCH, D], f32, tag="xh")
                    for lc in range(LCH):
                        nc.vector.tensor_scalar_mul(
                            out=xh[:, lc, :], in0=xs[:, lc, :],
                            scalar1=inv_xn[:, lc:lc + 1],
                        )
                    xhT = xio.tile([P128, DH, LSUP], f32, tag="xhT")
                    for dh in range(DH):
                        ptr = ppool_tr.tile([P128, LCH, P128], f32, tag="tr")
                        for lc in range(LCH):
                            nc.tensor.transpose(
                                ptr[:, lc, :],
                                xh[:, lc, dh * P128:(dh + 1) * P128], ident,
                            )
                        nc.scalar.copy(
                            xhT[:, dh, :], ptr.rearrange("p a b -> p (a b)")
                        )
                    return xhT

                xhT_next = prep(0)
                for st in range(NST):
                    L0 = st * LSUP
                    xhT = xhT_next
                    if st + 1 < NST:
                        xhT_next = prep(st + 1)
                    if st == NST - 2 and b + 1 < BS:
                        nextb = batch_prep(b + 1)

                    # dotsT[n, l] += phT[d, n].T @ xhT[d, l]; exp with the
                    # pad mask as a per-partition additive bias.
                    expT = wio.tile([P128, NCH, LSUP], f32, tag="expT")
                    for c in range(NCH):
                        dt = ppool_dt.tile([P128, LSUP], f32, tag="dt")
                        for dh in range(DH):
                            nc.tensor.matmul(
                                dt,
                                lhsT=phT[:, dh, c * P128:(c + 1) * P128],
                                rhs=xhT[:, dh, :],
                                start=(dh == 0), stop=(dh == DH - 1),
                            )
                        nc.scalar.activation(
                            out=expT[:, c, :], in_=dt, func=Act.Exp,
                            bias=mb_sb[:, c:c + 1], scale=1.0,
                        )

                    # softmax denominator: S[1, l] = sum_n expT[n, l]
                    Sp = ppool_s.tile([1, LSUP], f32, tag="S")
                    for c in range(NCH):
                        nc.tensor.matmul(
                            Sp, lhsT=ones_col, rhs=expT[:, c, :],
                            start=(c == 0), stop=(c == NCH - 1),
                        )
                    rS = small.tile([1, LSUP], f32, tag="rS")
                    nc.vector.reciprocal(out=rS, in_=Sp)
                    bc = wio.tile([P128, LSUP], f32, tag="bc")
                    nc.gpsimd.partition_broadcast(bc, rS)
                    wt = wio.tile([P128, NCH, LSUP], f32, tag="wt")
                    for c in range(NCH):
                        nc.gpsimd.tensor_mul(
                            out=wt[:, c, :], in0=expT[:, c, :], in1=bc
                        )
                    nc.sync.dma_start(
                        out=WT[b, :, L0:L0 + LSUP].rearrange(
                            "(c p) l -> p c l", p=P128
                        ),
                        in_=wt,
                    )

                    # emb_t[d, l] += prof[n, d].T @ expT[n, l]; the softmax
                    # denominator is folded into the PSUM->SBUF step so the
                    # PE never waits on the GpSimd normalize chain.
                    embs = wio.tile([P128, DH, LSUP], f32, tag="embs")
                    for dh in range(DH):
                        pe = ppool_e.tile([P128, LSUP], f32, tag="pe")
                        for c in range(NCH):
                            nc.tensor.matmul(
                                pe,
                                lhsT=prof[:, c, dh * P128:(dh + 1) * P128],
                                rhs=expT[:, c, :],
                                start=(c == 0), stop=(c == NCH - 1),
                            )
                        nc.vector.tensor_mul(out=embs[:, dh, :], in0=pe, in1=bc)
                    nc.sync.dma_start(
                        out=ET[b, :, L0:L0 + LSUP].rearrange(
                            "(dh p) l -> p dh l", p=P128
                        ),
                        in_=embs,
                    )

    nc.compile()
    return nc


def kernel(spk_decoder_out, profile, profile_lens):
    global _compiled
    from concourse.bass_utils import run_bass_kernel_spmd

    if _compiled is None:
        _compiled = _build()
    nc = _compiled

    x = np.ascontiguousarray(np.asarray(spk_decoder_out, dtype=np.float32))
    p = np.ascontiguousarray(np.asarray(profile, dtype=np.float32))
    lens = np.asarray(profile_lens).astype(np.int64)
    mask = np.arange(N, dtype=np.int64)[None, :] >= lens[:, None]
    mb = np.where(mask, np.float32(NEG), np.float32(0.0)).astype(np.float32)
    # device wants [partition, chunk] layout: (B, 128, NCH)
    mb = np.ascontiguousarray(mb.reshape(B, NCH, P128).transpose(0, 2, 1))

    in_maps = [
        {
            "x": x[i * BS:(i + 1) * BS],
            "profile": p[i * BS:(i + 1) * BS],
            "maskb": mb[i * BS:(i + 1) * BS],
        }
        for i in range(NCORES)
    ]
    res = run_bass_kernel_spmd(nc, in_maps, core_ids=list(range(NCORES)))
    wt = np.concatenate([r["w_t"] for r in res.results], axis=0)    # (B, N, L)
    et = np.concatenate([r["emb_t"] for r in res.results], axis=0)  # (B, D, L)
    weights = np.ascontiguousarray(wt.transpose(0, 2, 1))
    emb = np.ascontiguousarray(et.transpose(0, 2, 1))
    return emb, weights
